# revision 1
# baseline (speedup 1.0000x reference)
"""CFConv (SchNet continuous-filter convolution) on 8 TRN2 NeuronCores.

    h   = softplus(rbf @ w1 + b1)        # [N, NB, F]
    W   = h @ w2 + b2                    # [N, NB, F]
    out = sum_n x[neighbors] * W         # [N, F]

Sharding: atoms (dim 0) split 8 ways; x + filter weights replicated.
No collectives needed - each core gathers its neighbors from its own
full copy of x in DRAM.

Per-core dataflow (feature-major [f, pair] layout everywhere):
  mm1 (PE):   h_pre[f, p] = w1[r, f].T @ rbf_t[r, p]      -> PSUM f32
  ACT:        h = softplus(h_pre + b1)  (b1 = per-partition bias) -> SBUF fp16
  mm2 (PE):   W[f, p] = w2[g, f].T @ h[g, p]              -> PSUM f32
  SWDGE:      xjT[f, p] = x[nbr[p], f]  (dma_gather transpose=True)
  DVE STT:    prod = (W + b2) * xjT                       -> SBUF fp16
  DVE tree:   out[f, a] = sum over 32 neighbor columns (5 rounds of
              tensor_tensor adds in fp16 2x mode)
Output is [128 f, 2500 atoms] f32 per core; host transposes + concats.
"""

import os

import numpy as np

import concourse.bass as bass
import concourse.bacc as bacc
import concourse.mybir as mybir
import concourse.tile as tile
from contextlib import ExitStack

N_ATOMS = 20000
NB = 32
F = 128
R = 64
RK = R + 1                      # mm1 contraction rows: 64 rbf dims + ones row (b1)
NCORES = 8
NA = N_ATOMS // NCORES          # atoms per core     = 2500
NP = NA * NB                    # pairs per core     = 80000
CH = 1024                       # pairs per chunk (2 PSUM banks of f32)
SPAN = 4                        # chunks per span (gather/reduce granularity)

f16 = mybir.dt.float16
f32 = mybir.dt.float32
i16 = mybir.dt.int16

_CACHE = {}


def _chunks():
    """(offset, width) pairs covering [0, NP), width % 128 == 0."""
    out = []
    off = 0
    while off < NP:
        w = min(CH, NP - off)
        out.append((off, w))
        off += w
    return out


def _spans():
    """Group chunks into spans of up to SPAN chunks."""
    ch = _chunks()
    spans = []
    for i in range(0, len(ch), SPAN):
        group = ch[i : i + SPAN]
        s0 = group[0][0]
        sp = sum(w for _, w in group)
        spans.append((s0, sp, group))
    return spans


def _build():
    if "nc" in _CACHE:
        return _CACHE["nc"]
    nc = bacc.Bacc()

    x_d = nc.declare_dram_parameter("x", [N_ATOMS, F], f16, isOutput=False)
    rbf_d = nc.declare_dram_parameter("rbf_t", [RK, NP], f16, isOutput=False)
    idx_d = nc.declare_dram_parameter("idx", [128, NP // 16], i16, isOutput=False)
    w1_d = nc.declare_dram_parameter("w1", [RK, F], f16, isOutput=False)
    w2_d = nc.declare_dram_parameter("w2", [F, F], f16, isOutput=False)
    b2_d = nc.declare_dram_parameter("b2", [F, 1], f32, isOutput=False)
    out_d = nc.declare_dram_parameter("out", [F, NA], f32, isOutput=True)

    spans = _spans()
    max_sp = max(sp for _, sp, _ in spans)

    with tile.TileContext(nc) as tc, ExitStack() as ctx:
        consts = ctx.enter_context(tc.tile_pool(name="consts", bufs=1))
        hpool = ctx.enter_context(tc.tile_pool(name="hpool", bufs=3))
        spool = ctx.enter_context(tc.tile_pool(name="spool", bufs=2))
        ph_pool = ctx.enter_context(tc.tile_pool(name="ph", bufs=2, space="PSUM"))
        pw_pool = ctx.enter_context(tc.tile_pool(name="pw", bufs=2, space="PSUM"))

        w1s = consts.tile([RK, F], f16)
        nc.sync.dma_start(out=w1s, in_=w1_d[:])
        w2s = consts.tile([F, F], f16)
        nc.sync.dma_start(out=w2s, in_=w2_d[:])
        b2s = consts.tile([F, 1], f32)
        nc.sync.dma_start(out=b2s, in_=b2_d[:])
        # DVE-local copy of b2: the STT multiply runs on DVE, so reading a
        # DVE-written tile adds no cross-engine sync wait.
        b2v = consts.tile([F, 1], f32)
        nc.vector.tensor_copy(out=b2v, in_=b2s)
        idxs = consts.tile([128, NP // 16], i16)
        nc.sync.dma_start(out=idxs, in_=idx_d[:])
        outst = consts.tile([F, NA], f32)

        for s0, sp, group in spans:
            atom0 = s0 // NB
            natoms = sp // NB

            rbft = spool.tile([RK, max_sp], f16, tag="rbft")
            nc.sync.dma_start(out=rbft[:, :sp], in_=rbf_d[:, s0 : s0 + sp])

            xjt = spool.tile([128, max_sp], f16, tag="xjt")
            nc.gpsimd.dma_gather(
                xjt[:, :sp].rearrange("p (o n) -> p o n", o=1),
                x_d[:],
                idxs[:, s0 // 16 : (s0 + sp) // 16],
                sp,
                sp,
                F,
                transpose=True,
                # single_packet=True crashes the SDMA engine above ~512
                # indices (NRT_EXEC_UNIT_UNRECOVERABLE) - packets are capped
                # at 64 descriptors.
                single_packet=False,
            )

            prod = spool.tile([128, max_sp], f16, tag="prod")
            es = spool.tile([128, max_sp], f16, tag="es")

            # pass 1 per chunk: e = exp(h_pre + b1)   (PSUM -> SBUF fp16)
            for off, w in group:
                co = off - s0  # offset within span
                ph = ph_pool.tile([128, CH], f32)
                # mm1 in <=512-column pieces (one PSUM bank each)
                for o in range(0, w, 512):
                    n = min(512, w - o)
                    nc.tensor.matmul(
                        ph[:, o : o + n],
                        w1s[:],
                        rbft[:, co + o : co + o + n],
                        start=True,
                        stop=True,
                    )
                nc.scalar.activation(
                    out=es[:, co : co + w],
                    in_=ph[:, :w],
                    func=mybir.ActivationFunctionType.Exp,
                    bias=0.0,
                    scale=1.0,
                )

            # pass 2 per span: h = softplus = ln(e + 1)  (one big SBUF op;
            # exp and ln share one ACT table set -> no table switches)
            hsp = spool.tile([128, max_sp], f16, tag="hsp")
            nc.scalar.activation(
                out=hsp[:, :sp],
                in_=es[:, :sp],
                func=mybir.ActivationFunctionType.Ln,
                bias=1.0,
                scale=1.0,
            )

            for off, w in group:
                co = off - s0
                pw = pw_pool.tile([128, CH], f32)
                for o in range(0, w, 512):
                    n = min(512, w - o)
                    nc.tensor.matmul(
                        pw[:, o : o + n],
                        w2s[:],
                        hsp[:, co + o : co + o + n],
                        start=True,
                        stop=True,
                    )
                # prod = (W + b2) * xjT  (fused bias + multiply, PSUM read)
                nc.vector.scalar_tensor_tensor(
                    out=prod[:, co : co + w],
                    in0=pw[:, :w],
                    scalar=b2v[:, 0:1],
                    in1=xjt[:, co : co + w],
                    op0=mybir.AluOpType.add,
                    op1=mybir.AluOpType.mult,
                )

            # neighbor reduction: binary tree over the 32 columns per atom
            pv = prod[:, :sp].rearrange("p (a n) -> p a n", n=NB)
            red1 = spool.tile([128, max_sp // 2], f16, tag="red1")
            r1 = red1[:, : sp // 2].rearrange("p (a n) -> p a n", n=16)
            nc.vector.tensor_tensor(
                out=r1, in0=pv[:, :, 0:16], in1=pv[:, :, 16:32],
                op=mybir.AluOpType.add,
            )
            red2 = spool.tile([128, max_sp // 4], f16, tag="red2")
            r2 = red2[:, : sp // 4].rearrange("p (a n) -> p a n", n=8)
            nc.vector.tensor_tensor(
                out=r2, in0=r1[:, :, 0:8], in1=r1[:, :, 8:16],
                op=mybir.AluOpType.add,
            )
            red3 = spool.tile([128, max_sp // 8], f16, tag="red3")
            r3 = red3[:, : sp // 8].rearrange("p (a n) -> p a n", n=4)
            nc.vector.tensor_tensor(
                out=r3, in0=r2[:, :, 0:4], in1=r2[:, :, 4:8],
                op=mybir.AluOpType.add,
            )
            red4 = spool.tile([128, max_sp // 16], f16, tag="red4")
            r4 = red4[:, : sp // 16].rearrange("p (a n) -> p a n", n=2)
            nc.vector.tensor_tensor(
                out=r4, in0=r3[:, :, 0:2], in1=r3[:, :, 2:4],
                op=mybir.AluOpType.add,
            )
            nc.vector.tensor_tensor(
                out=outst[:, atom0 : atom0 + natoms].rearrange(
                    "p (a o) -> p a o", o=1
                ),
                in0=r4[:, :, 0:1],
                in1=r4[:, :, 1:2],
                op=mybir.AluOpType.add,
            )

        nc.sync.dma_start(out=out_d[:], in_=outst[:])

    # Bacc.finalize() runs the sync-wait legalization (each TRN2 instruction
    # carries at most one wait; extras are split into event-semaphore insts).
    nc.finalize()
    _CACHE["nc"] = nc
    return nc


def _prep_core_inputs(x16, rbf, neighbors, w1a_16, w2_16, b2c, c):
    a0, a1 = c * NA, (c + 1) * NA
    rbf_t = np.empty((RK, NP), dtype=np.float16)
    rbf_t[:R] = rbf[a0:a1].reshape(NP, R).T
    rbf_t[R] = 1.0  # ones row: contracts with the b1 row of w1a
    nb = np.ascontiguousarray(neighbors[a0:a1]).reshape(NP).astype(np.int16)
    # dma_gather index layout: element i lives at [i % 16, i // 16],
    # 16-partition block replicated 8x down the partition dim.
    idx16 = np.ascontiguousarray(nb.reshape(NP // 16, 16).T)
    idx = np.tile(idx16, (8, 1))
    return {
        "x": x16,
        "rbf_t": rbf_t,
        "idx": np.ascontiguousarray(idx),
        "w1": w1a_16,
        "w2": w2_16,
        "b2": b2c,
    }


def kernel(x, rbf, neighbors, w1, b1, w2, b2):
    from concourse.bass_utils import run_bass_kernel_spmd

    x = np.asarray(x)
    rbf = np.asarray(rbf)
    neighbors = np.asarray(neighbors)
    w1 = np.asarray(w1)
    b1 = np.asarray(b1)
    w2 = np.asarray(w2)
    b2 = np.asarray(b2)

    nc = _build()

    x16 = x.astype(np.float16)
    w1a_16 = np.ascontiguousarray(
        np.vstack([w1, b1.reshape(1, F)]).astype(np.float16)
    )
    w2_16 = w2.astype(np.float16)
    b2c = np.ascontiguousarray(b2.reshape(F, 1).astype(np.float32))

    in_maps = [
        _prep_core_inputs(x16, rbf, neighbors, w1a_16, w2_16, b2c, c)
        for c in range(NCORES)
    ]

    res = run_bass_kernel_spmd(
        nc,
        in_maps,
        core_ids=list(range(NCORES)),
        trace=bool(int(os.environ.get("CFCONV_TRACE", "0"))),
    )
    _CACHE["last_result"] = res

    out = np.concatenate(
        [res.results[c]["out"].T for c in range(NCORES)], axis=0
    )
    return np.ascontiguousarray(out.astype(np.float32))



# revision 6
# speedup vs baseline: 1.2612x; 1.2612x over previous
"""CFConv (SchNet continuous-filter convolution) on 8 TRN2 NeuronCores.

    h   = softplus(rbf @ w1 + b1)        # [N, NB, F]
    W   = h @ w2 + b2                    # [N, NB, F]
    out = sum_n x[neighbors] * W         # [N, F]

Sharding: atoms (dim 0) split 8 ways; x + filter weights replicated.
No collectives needed - each core gathers its neighbors from its own
full copy of x in DRAM.

Per-core dataflow (feature-major [f, pair] layout everywhere):
  mm1 (PE):   h_pre[f, p] = w1[r, f].T @ rbf_t[r, p]      -> PSUM f32
  ACT:        h = softplus(h_pre + b1)  (native Softplus, b1 = bias AP)
  mm2 (PE):   W[f, p] = w2[g, f].T @ h[g, p]              -> PSUM f32
  SWDGE:      xjT[f, p] = x[nbr[p], f]  (dma_gather transpose=True,
              4-way split across SWDGE queues 0-3 so all four Q7 core
              pairs generate descriptors concurrently)
  DVE STT:    prod = (W + b2) * xjT                       -> SBUF fp16
  DVE reduce: out[f, a] = tensor_reduce over the 32-neighbor axis
Output is [128 f, 2500 atoms] f32 per core; host transposes + concats.
"""

import os

import numpy as np

import concourse.bass as bass
import concourse.bacc as bacc
import concourse.mybir as mybir
import concourse.tile as tile
from contextlib import ExitStack

N_ATOMS = 20000
NB = 32
F = 128
R = 64
NCORES = 8
NA = N_ATOMS // NCORES          # atoms per core     = 2500
NP = NA * NB                    # pairs per core     = 80000
CH = 1024                       # pairs per chunk (2 PSUM banks of f32)
SPAN = 4                        # chunks per span (gather/reduce granularity)
NQ = 4                          # SWDGE queues (Q7 core pairs) for gathers

f16 = mybir.dt.float16
f32 = mybir.dt.float32
i16 = mybir.dt.int16

_CACHE = {}


class _Bacc(bacc.Bacc):
    """Bacc with exp+ln pinned to the one act-func table containing both.

    The stock placement pass picks the first act_info.json set containing
    each activation function: Exp -> "exp_and_others", Ln -> "natural_log",
    which alternates ACT_TABLE_LOADs (~1.5us each) on every exp/ln switch.
    Removing Exp/Ln from every other set (list order preserved, so the
    positional act_func_set_id stays valid) pins both to
    "natural_log_exp_and_others" -> one load for the whole kernel.
    """

    def insert_act_table_loads(self):
        from concourse.hw_specs import get_activation_tables
        from concourse.bacc import _bass_rust

        has_activation = any(
            isinstance(i, mybir.InstActivation)
            for b in self.main_func.blocks
            for i in b.instructions
        )
        if not has_activation:
            return
        tables = list(get_activation_tables(self.m.arch).items())
        shared = {mybir.ActivationFunctionType.Exp, mybir.ActivationFunctionType.Ln}
        tables = [
            (name, s if name == "natural_log_exp_and_others" else (s - shared))
            for name, s in tables
        ]
        _bass_rust.insert_act_table_loads(self, tables)


def _chunks():
    """(offset, width) pairs covering [0, NP), width % 128 == 0."""
    out = []
    off = 0
    while off < NP:
        w = min(CH, NP - off)
        out.append((off, w))
        off += w
    return out


def _spans():
    """Group chunks into spans of up to SPAN chunks."""
    ch = _chunks()
    spans = []
    for i in range(0, len(ch), SPAN):
        group = ch[i : i + SPAN]
        s0 = group[0][0]
        sp = sum(w for _, w in group)
        spans.append((s0, sp, group))
    return spans


def _qsplit(sp):
    """Split sp into NQ (offset, width) parts, each width % 128 == 0."""
    if os.environ.get("CFCONV_1Q"):
        return [(0, sp)]
    n128 = sp // 128
    base = n128 // NQ
    rem = n128 % NQ
    parts = []
    off = 0
    for q in range(NQ):
        w = (base + (1 if q < rem else 0)) * 128
        if w:
            parts.append((off, w))
        off += w
    return parts


def _build():
    if "nc" in _CACHE:
        return _CACHE["nc"]
    nc = _Bacc(num_swdge_queues=NQ)

    x_d = nc.declare_dram_parameter("x", [N_ATOMS, F], f16, isOutput=False)
    rbf_d = nc.declare_dram_parameter("rbf_t", [R, NP], f16, isOutput=False)
    idx_d = nc.declare_dram_parameter("idx", [128, NP // 16], i16, isOutput=False)
    w1_d = nc.declare_dram_parameter("w1", [R, F], f16, isOutput=False)
    w2_d = nc.declare_dram_parameter("w2", [F, F], f16, isOutput=False)
    b1_d = nc.declare_dram_parameter("b1", [F, 1], f32, isOutput=False)
    b2_d = nc.declare_dram_parameter("b2", [F, 1], f32, isOutput=False)
    out_d = nc.declare_dram_parameter("out", [F, NA], f32, isOutput=True)

    spans = _spans()
    max_sp = max(sp for _, sp, _ in spans)

    with tile.TileContext(nc) as tc, ExitStack() as ctx:
        consts = ctx.enter_context(tc.tile_pool(name="consts", bufs=1))
        spool = ctx.enter_context(tc.tile_pool(name="spool", bufs=2))
        ph_pool = ctx.enter_context(tc.tile_pool(name="ph", bufs=2, space="PSUM"))
        pw_pool = ctx.enter_context(tc.tile_pool(name="pw", bufs=2, space="PSUM"))

        w1s = consts.tile([R, F], f16)
        nc.sync.dma_start(out=w1s, in_=w1_d[:])
        w2s = consts.tile([F, F], f16)
        nc.sync.dma_start(out=w2s, in_=w2_d[:])
        b1s = consts.tile([F, 1], f32)
        nc.sync.dma_start(out=b1s, in_=b1_d[:])
        b2s = consts.tile([F, 1], f32)
        nc.sync.dma_start(out=b2s, in_=b2_d[:])
        # DVE-local copy of b2: the STT multiply runs on DVE, so reading a
        # DVE-written tile adds no cross-engine sync wait.
        b2v = consts.tile([F, 1], f32)
        nc.vector.tensor_copy(out=b2v, in_=b2s)
        idxs = consts.tile([128, NP // 16], i16)
        nc.sync.dma_start(out=idxs, in_=idx_d[:])
        outst = consts.tile([F, NA], f32)

        for s0, sp, group in spans:
            atom0 = s0 // NB
            natoms = sp // NB

            rbft = spool.tile([R, max_sp], f16, tag="rbft")
            nc.sync.dma_start(out=rbft[:, :sp], in_=rbf_d[:, s0 : s0 + sp])

            xjt = spool.tile([128, max_sp], f16, tag="xjt")
            # 4-way gather: each SWDGE queue q runs on Q7 core pair
            # (2q, 2q+1), so the four descriptor generations overlap.
            for q, (qoff, qw) in enumerate(_qsplit(sp)):
                nc.gpsimd.dma_gather(
                    xjt[:, qoff : qoff + qw].rearrange("p (o n) -> p o n", o=1),
                    x_d[:],
                    idxs[:, (s0 + qoff) // 16 : (s0 + qoff + qw) // 16],
                    qw,
                    qw,
                    F,
                    transpose=True,
                    # single_packet=True crashes the SDMA engine above ~512
                    # indices (NRT_EXEC_UNIT_UNRECOVERABLE) - packets are
                    # capped at 64 descriptors.
                    single_packet=False,
                    queue_num=q,
                )

            prod = spool.tile([128, max_sp], f16, tag="prod")
            es = spool.tile([128, max_sp], f16, tag="es")

            # pass 1 per chunk: e = exp(h_pre + b1)   (PSUM -> SBUF fp16;
            # b1 folded in as the ACT bias AP)
            for off, w in group:
                co = off - s0  # offset within span
                ph = ph_pool.tile([128, CH], f32)
                # mm1 in <=512-column pieces (one PSUM bank each)
                for o in range(0, w, 512):
                    n = min(512, w - o)
                    nc.tensor.matmul(
                        ph[:, o : o + n],
                        w1s[:],
                        rbft[:, co + o : co + o + n],
                        start=True,
                        stop=True,
                    )
                nc.scalar.activation(
                    out=es[:, co : co + w],
                    in_=ph[:, :w],
                    func=mybir.ActivationFunctionType.Exp,
                    bias=b1s[:, 0:1],
                    scale=1.0,
                )

            # pass 2 per span: h = softplus = ln(e + 1)  (one big SBUF op;
            # both exp and ln resolve to the natural_log_exp_and_others
            # table -> no table switches)
            hsp = spool.tile([128, max_sp], f16, tag="hsp")
            nc.scalar.activation(
                out=hsp[:, :sp],
                in_=es[:, :sp],
                func=mybir.ActivationFunctionType.Ln,
                bias=1.0,
                scale=1.0,
            )

            for off, w in group:
                co = off - s0
                pw = pw_pool.tile([128, CH], f32)
                for o in range(0, w, 512):
                    n = min(512, w - o)
                    nc.tensor.matmul(
                        pw[:, o : o + n],
                        w2s[:],
                        hsp[:, co + o : co + o + n],
                        start=True,
                        stop=True,
                    )
                # prod = (W + b2) * xjT  (fused bias + multiply, PSUM read)
                nc.vector.scalar_tensor_tensor(
                    out=prod[:, co : co + w],
                    in0=pw[:, :w],
                    scalar=b2v[:, 0:1],
                    in1=xjt[:, co : co + w],
                    op0=mybir.AluOpType.add,
                    op1=mybir.AluOpType.mult,
                )

            # neighbor reduction: one segmented reduce over the 32-column
            # groups, straight into the fp32 output staging tile.
            nc.vector.tensor_reduce(
                out=outst[:, atom0 : atom0 + natoms],
                in_=prod[:, :sp].rearrange("p (a n) -> p a n", n=NB),
                axis=mybir.AxisListType.X,
                op=mybir.AluOpType.add,
            )

        nc.sync.dma_start(out=out_d[:], in_=outst[:])

    # Bacc.finalize() runs the sync-wait legalization (each TRN2 instruction
    # carries at most one wait; extras are split into event-semaphore insts).
    nc.finalize()
    _CACHE["nc"] = nc
    return nc


def _prep_core_inputs(x16, rbf, neighbors, w1_16, w2_16, b1c, b2c, c):
    a0, a1 = c * NA, (c + 1) * NA
    rbf_t = np.ascontiguousarray(rbf[a0:a1].reshape(NP, R).T.astype(np.float16))
    nb = np.ascontiguousarray(neighbors[a0:a1]).reshape(NP).astype(np.int16)
    # dma_gather index layout: element i lives at [i % 16, i // 16],
    # 16-partition block replicated 8x down the partition dim.
    idx16 = np.ascontiguousarray(nb.reshape(NP // 16, 16).T)
    idx = np.tile(idx16, (8, 1))
    return {
        "x": x16,
        "rbf_t": rbf_t,
        "idx": np.ascontiguousarray(idx),
        "w1": w1_16,
        "w2": w2_16,
        "b1": b1c,
        "b2": b2c,
    }


def kernel(x, rbf, neighbors, w1, b1, w2, b2):
    from concourse.bass_utils import run_bass_kernel_spmd

    x = np.asarray(x)
    rbf = np.asarray(rbf)
    neighbors = np.asarray(neighbors)
    w1 = np.asarray(w1)
    b1 = np.asarray(b1)
    w2 = np.asarray(w2)
    b2 = np.asarray(b2)

    nc = _build()

    x16 = x.astype(np.float16)
    w1_16 = np.ascontiguousarray(w1.astype(np.float16))
    w2_16 = w2.astype(np.float16)
    b1c = np.ascontiguousarray(b1.reshape(F, 1).astype(np.float32))
    b2c = np.ascontiguousarray(b2.reshape(F, 1).astype(np.float32))

    in_maps = [
        _prep_core_inputs(x16, rbf, neighbors, w1_16, w2_16, b1c, b2c, c)
        for c in range(NCORES)
    ]

    res = run_bass_kernel_spmd(
        nc,
        in_maps,
        core_ids=list(range(NCORES)),
        trace=bool(int(os.environ.get("CFCONV_TRACE", "0"))),
    )
    _CACHE["last_result"] = res

    out = np.concatenate(
        [res.results[c]["out"].T for c in range(NCORES)], axis=0
    )
    return np.ascontiguousarray(out.astype(np.float32))
